# revision 1
# baseline (speedup 1.0000x reference)
"""DLEM kernel for Trainium2, 8 NeuronCores, data-parallel over batch.

Pipeline per core (8 samples):
  1. Conv section on the PE (float32r matmuls):
     conv0(160->128) and conv1(96->96) per sample, conv2(64->64) 2-sample
     block-diag, conv3(32->32) + all transposed convs 4-sample block-diag,
     mixer folded into 4 sparse [128,8] matmuls. PSUM->SBUF evacuation with
     fused bias+relu on ACT/DVE.
  2. DLEM scan rewritten as a linear 2-term recurrence without division:
         u_{d+1}[j] = u_d[j] - a_d[j] + a_d[j+1],  a_d = P_d * u_d,
         P(d,j) = right[j] / (right[j] + left[d+j])
     (u = curr * den / const^{d-1}; const and all divisions fold into the
     P coefficients and one final fixup.)
     Layout: 128 partitions = (chunk c)*8 + sample s, 16 chunks of 128
     positions + 64-deep halo; halo refreshed by one SBUF DMA per 64 steps.
     3 stock DVE tensor_tensor ops per step; P built in bulk per phase
     (den on GPSIMD, reciprocal on DVE, multiply on GPSIMD).
"""
import os
import sys

for _p in ("/opt/trn_rl_repo", "/root/.axon_site/_ro/trn_rl_repo"):
    if os.path.isdir(_p) and _p not in sys.path:
        sys.path.insert(0, _p)

import numpy as np

B = 64
NCORES = 8
S = B // NCORES          # samples per core
N = 2048
EPI, SEQD, CPR = 128, 32, 32
NSTEP = 510              # d = 1..510
RH = 64                  # halo depth / phase length
FD = 128 + RH            # scan tile free dim
C = 16                   # chunks per sample
NPH = (NSTEP + RH - 1) // RH   # 8 phases (last = 62 steps)
LEXT = 2700              # left_ext width  (needs 1920+703=2623)
GEXT = C * 128 + RH      # 2112, right/u width
LROW = FD + 511          # 703, left_row width

F32R = True              # use float32r (1 cyc/row) matmuls
XSPLIT = 120             # seq-step DVE/Pool free-dim split (DVE gets [0, XSPLIT))

_LBR = (2046, 2044, 2042, 2040)   # branch initial lengths
_T0S = (3, 4, 5, 6)               # branch initial data col offsets


# ---------------------------------------------------------------- weights --
def _pack_weights(inp):
    """All numpy-side weight packing. lhsT tensors are (K, 3, M)."""
    d = {}
    w0 = inp['conv0_w']            # (128, 160, 3)
    perm0 = np.concatenate([np.arange(32, 128), np.arange(0, 32)])
    d['c0A'] = np.ascontiguousarray(np.transpose(w0[perm0][:, :128, :], (1, 2, 0)))
    # c0B: K = (k, seq-ch) stacked 96 rows for the shift-stacked seq tile
    c0b_stk = np.zeros((96, 128), np.float32)
    for k in range(3):
        c0b_stk[32 * k:32 * k + 32, :] = w0[perm0][:, 128:, k].T
    d['c0B'] = c0b_stk
    d['c0bl'] = inp['conv0_b'][0:32].reshape(32, 1)
    d['c0bh'] = inp['conv0_b'][32:128].reshape(96, 1)

    w1 = inp['conv1_w']            # (96, 96, 3)
    perm1 = np.concatenate([np.arange(32, 96), np.arange(0, 32)])
    d['c1'] = np.ascontiguousarray(np.transpose(w1[perm1], (1, 2, 0)))
    d['c1bl'] = inp['conv1_b'][0:32].reshape(32, 1)
    d['c1bh'] = inp['conv1_b'][32:96].reshape(64, 1)

    w2 = inp['conv2_w']            # (64, 64, 3)
    l2 = np.zeros((128, 3, 128), np.float32)
    for k in range(3):
        t = w2[:, :, k].T          # (ci, co)
        # even sample: k-rows 0:64; odd: 64:128
        # out cols: 0:32 evenS co0:32 | 32:64 oddS co0:32
        #           64:96 evenS co32:64 | 96:128 oddS co32:64
        l2[0:64, k, 0:32] = t[:, 0:32]
        l2[0:64, k, 64:96] = t[:, 32:64]
        l2[64:128, k, 32:64] = t[:, 0:32]
        l2[64:128, k, 96:128] = t[:, 32:64]
    d['c2'] = l2
    b2 = np.zeros((128, 1), np.float32)
    b2[0:32, 0] = inp['conv2_b'][0:32]
    b2[32:64, 0] = inp['conv2_b'][0:32]
    b2[64:96, 0] = inp['conv2_b'][32:64]
    b2[96:128, 0] = inp['conv2_b'][32:64]
    d['c2bl'] = b2[0:64]
    d['c2bh'] = b2[64:128]

    w3 = inp['conv3_w']            # (32, 32, 3)
    l3 = np.zeros((128, 3, 128), np.float32)
    for k in range(3):
        t = w3[:, :, k].T
        for sl in range(4):
            l3[32 * sl:32 * sl + 32, k, 32 * sl:32 * sl + 32] = t
    d['c3'] = l3
    d['c3b'] = np.tile(inp['conv3_b'], 4).reshape(128, 1)

    for i in range(4):
        w = inp[f'tconv{i}_w']     # (ci=32, co=32, 3) torch layout
        wf = np.flip(w, -1).transpose(1, 0, 2)   # (co, ci, 3)
        lt = np.zeros((128, 3, 128), np.float32)
        for k in range(3):
            t = wf[:, :, k].T      # (ci, co)
            for sl in range(4):
                lt[32 * sl:32 * sl + 32, k, 32 * sl:32 * sl + 32] = t
        d[f't{i}'] = lt
        d[f't{i}b'] = np.tile(inp[f'tconv{i}_b'], 4).reshape(128, 1)

    mw = inp['mixer_w'][:, :, 0]   # (2, 128)
    for i in range(4):
        m = np.zeros((128, 36), np.float32)
        for sl in range(4):
            m[32 * sl:32 * sl + 32, sl] = mw[0, 32 * i:32 * i + 32]      # left
            m[32 * sl:32 * sl + 32, 32 + sl] = mw[1, 32 * i:32 * i + 32]  # right
        d[f'mx{i}'] = m
    d['mxbl'] = np.full((4, 1), inp['mixer_b'][0], np.float32)
    d['mxbr'] = np.full((4, 1), inp['mixer_b'][1], np.float32)
    return d


_WSHAPES = {'c0A': [128, 3, 128],
            'c0bl': [32, 1], 'c0bh': [96, 1], 'c0B': [96, 128],
            'c1': [96, 3, 96], 'c1bl': [32, 1], 'c1bh': [64, 1],
            'c2': [128, 3, 128], 'c2bl': [64, 1], 'c2bh': [64, 1],
            'c3': [128, 3, 128], 'c3b': [128, 1],
            'mxbl': [4, 1], 'mxbr': [4, 1]}
for _i in range(4):
    _WSHAPES[f't{_i}'] = [128, 3, 128]
    _WSHAPES[f't{_i}b'] = [128, 1]
    _WSHAPES[f'mx{_i}'] = [128, 36]


def coltiles(L, maxw=512):
    out, off = [], 0
    while off < L:
        w = min(maxw, L - off)
        out.append((off, w))
        off += w
    return out


# ---------------------------------------------------------------- program --
def build_program(const_val):
    import bass_rust
    import concourse.bacc as bacc
    import concourse.mybir as mybir
    from concourse.tile import TileContext

    f32 = mybir.dt.float32
    f32r = mybir.dt.float32r
    AF = mybir.ActivationFunctionType
    OP = mybir.AluOpType

    def cap(ap, dims, offset=None):
        b = ap.copy()
        b.ap = bass_rust.VecI64Pair(dims)
        if offset is not None:
            b.offset = offset
        return b

    nc = bacc.Bacc("TRN2", target_bir_lowering=False, debug=False,
                   num_devices=NCORES)

    sig = nc.declare_dram_parameter("signal", [S, EPI, N], f32r, isOutput=False)
    seq = nc.declare_dram_parameter("seq", [S, SEQD, N], f32r, isOutput=False)
    curr = nc.declare_dram_parameter("curr", [S, N - 1], f32, isOutput=False)
    wd = {k: nc.declare_dram_parameter(
              k, sh, f32 if (k.endswith('b') or 'bl' in k or 'bh' in k
                             or k in ('mxbl', 'mxbr')) else f32r,
              isOutput=False)
          for k, sh in _WSHAPES.items()}
    yout = nc.declare_dram_parameter("y", [S, 1537], f32, isOutput=True)

    L0, L1, L2, L3 = _LBR[0], _LBR[1], _LBR[2], _LBR[3]
    BW = 2052

    with TileContext(nc) as tc:
        with (tc.tile_pool(name="wp", bufs=1) as wp,
              tc.tile_pool(name="lrp", bufs=1) as lrp,
              tc.tile_pool(name="psp", bufs=2, space="PSUM") as psp):

            wt = {}
            for k, sh in _WSHAPES.items():
                t = wp.tile(sh,
                            f32 if (k.endswith('b') or 'bl' in k or 'bh' in k
                                    or k in ('mxbl', 'mxbr')) else f32r,
                            name=f"w_{k}", tag=f"w_{k}")
                nc.scalar.dma_start(out=t[:], in_=wd[k][:])
                wt[k] = t

            # long-lived scan-side tiles (coexist with conv tiles)
            left_ext = lrp.tile([36, LEXT], f32, tag="left_ext")    # g0 rows 0:4, g1 rows 32:36
            right_ext = lrp.tile([36, GEXT], f32, tag="right_ext")
            curr1 = lrp.tile([36, GEXT], f32, tag="curr1")
            u_full = lrp.tile([36, GEXT], f32, tag="u_full")
            u_t = lrp.tile([128, FD], f32, tag="u_t")
            r_row = lrp.tile([128, FD], f32, tag="r_row")
            l_row = lrp.tile([128, LROW], f32, tag="l_row")
            nc.gpsimd.memset(left_ext[:], 1.0)
            nc.gpsimd.memset(right_ext[:], 0.0)
            nc.gpsimd.memset(curr1[:], 0.0)
            for g in range(2):
                nc.scalar.dma_start(out=curr1[32 * g:32 * g + 4, 0:N - 1],
                                  in_=curr[4 * g:4 * g + 4])

            # ------------------------ conv section ------------------------
            with (tc.tile_pool(name="actp", bufs=1) as actp,
                  tc.tile_pool(name="iop", bufs=2) as iop):
                for g in range(2):
                    b_init = [actp.tile([128, BW], f32r, name=f"binit{i}", tag=f"binit{i}")
                              for i in range(4)]
                    b_alt = [actp.tile([128, BW], f32r, name=f"balt{i}", tag=f"balt{i}")
                             for i in range(4)]
                    zsrc = wt['c3'][:, 0, 0:8]
                    for i in range(4):
                        for t in (b_init[i], b_alt[i]):
                            nc.scalar.activation(t[:, 0:8], zsrc, AF.Copy,
                                                 bias=0.0, scale=0.0)
                            nc.scalar.activation(t[:, BW - 8:BW], zsrc, AF.Copy,
                                                 bias=0.0, scale=0.0)

                    conv2in = [actp.tile([128, L1], f32r, name=f"c2in{p}", tag=f"c2in{p}")
                               for p in range(2)]
                    conv3in = actp.tile([128, L2], f32r, tag="c3in")

                    def evac(ci, dst, src, bias, act_first=True):
                        """relu(src + bias) -> dst, alternating ACT/DVE."""
                        if (ci % 2 == 0) == act_first:
                            nc.scalar.activation(dst, src, AF.Relu,
                                                 bias=bias, scale=1.0)
                        else:
                            nc.vector.tensor_scalar(dst, src, bias, 0.0,
                                                    OP.add, OP.max)

                    for sl in range(4):
                        s = 4 * g + sl
                        x0a = iop.tile([128, N], f32r, tag="x0a")
                        nc.sync.dma_start(out=x0a[:], in_=sig[s])
                        # shift-stacked seq tile: rows (k, c) = seq[s, c, f+k]
                        seq_stk = iop.tile([96, L0], f32r, tag="seq_stk")
                        for k in range(3):
                            nc.sync.dma_start(out=seq_stk[32 * k:32 * k + 32, :],
                                              in_=seq[s][:, k:k + L0])
                        rest1 = iop.tile([96, L0], f32r, tag="rest1")

                        # conv0: 160->128 (3 sig shifts + 1 stacked-seq matmul)
                        for ci, (off, w) in enumerate(coltiles(L0)):
                            ps = psp.tile([128, 512], f32, tag="ps")
                            for k in range(3):
                                nc.tensor.matmul(
                                    out=ps[:, 0:w], lhsT=(wt['c0A'][:, k, :]),
                                    rhs=(x0a[:, off + k:off + k + w]),
                                    start=(k == 0), stop=False)
                            nc.tensor.matmul(
                                out=ps[:, 0:w], lhsT=(wt['c0B'][:]),
                                rhs=(seq_stk[:, off:off + w]),
                                start=False, stop=True)
                            evac(ci, b_init[0][32 * sl:32 * sl + 32,
                                               3 + off:3 + off + w],
                                 ps[96:128, 0:w], wt['c0bl'][:])
                            evac(ci + 1, rest1[:, off:off + w],
                                 ps[0:96, 0:w], wt['c0bh'][:])

                        # conv1: 96->96
                        for ci, (off, w) in enumerate(coltiles(L1)):
                            ps1 = psp.tile([96, 512], f32, tag="ps1")
                            for k in range(3):
                                nc.tensor.matmul(
                                    out=ps1[:, 0:w], lhsT=(wt['c1'][:, k, :]),
                                    rhs=(rest1[:, off + k:off + k + w]),
                                    start=(k == 0), stop=(k == 2))
                            evac(ci, b_init[1][32 * sl:32 * sl + 32,
                                               4 + off:4 + off + w],
                                 ps1[64:96, 0:w], wt['c1bl'][:])
                            evac(ci + 1,
                                 conv2in[sl // 2][64 * (sl % 2):64 * (sl % 2) + 64,
                                                  off:off + w],
                                 ps1[0:64, 0:w], wt['c1bh'][:])

                    # conv2 per pair (2 samples block-diag)
                    for p in range(2):
                        for ci, (off, w) in enumerate(coltiles(L2)):
                            ps = psp.tile([128, 512], f32, tag="ps")
                            for k in range(3):
                                nc.tensor.matmul(
                                    out=ps[:, 0:w], lhsT=(wt['c2'][:, k, :]),
                                    rhs=(conv2in[p][:, off + k:off + k + w]),
                                    start=(k == 0), stop=(k == 2))
                            evac(ci, b_init[2][64 * p:64 * p + 64,
                                               5 + off:5 + off + w],
                                 ps[0:64, 0:w], wt['c2bl'][:])
                            evac(ci + 1, conv3in[64 * p:64 * p + 64, off:off + w],
                                 ps[64:128, 0:w], wt['c2bh'][:])

                    # conv3 (4-sample block-diag)
                    for ci, (off, w) in enumerate(coltiles(L3)):
                        ps = psp.tile([128, 512], f32, tag="ps")
                        for k in range(3):
                            nc.tensor.matmul(
                                out=ps[:, 0:w], lhsT=(wt['c3'][:, k, :]),
                                rhs=(conv3in[:, off + k:off + k + w]),
                                start=(k == 0), stop=(k == 2))
                        evac(ci, b_init[3][:, 6 + off:6 + off + w],
                             ps[:, 0:w], wt['c3b'][:])

                    # transposed conv chains (4-sample block-diag)
                    finals = []
                    for i in range(4):
                        t0, L = _T0S[i], _LBR[i]
                        src, dst = b_init[i], b_alt[i]
                        for ti in range(3 - i, 4):
                            Lo = L + 2
                            for ci, (off, w) in enumerate(coltiles(Lo)):
                                ps = psp.tile([128, 512], f32, tag="ps")
                                for k in range(3):
                                    nc.tensor.matmul(
                                        out=ps[:, 0:w],
                                        lhsT=(wt[f't{ti}'][:, k, :]),
                                        rhs=(src[:, t0 - 2 + off + k:
                                                 t0 - 2 + off + k + w]),
                                        start=(k == 0), stop=(k == 2))
                                evac(ci, dst[:, t0 - 1 + off:t0 - 1 + off + w],
                                     ps[:, 0:w], wt[f't{ti}b'][:])
                            src, dst = dst, src
                            t0, L = t0 - 1, L + 2
                        finals.append(src)   # final 2048 cols at [2, 2050)

                    # mixer + sigmoid -> left/right rows directly
                    for (off, w) in coltiles(N):
                        psm = psp.tile([36, 512], f32, tag="psm")
                        for i in range(4):
                            nc.tensor.matmul(
                                out=psm[:, 0:w], lhsT=(wt[f'mx{i}'][:]),
                                rhs=(finals[i][:, 2 + off:2 + off + w]),
                                start=(i == 0), stop=(i == 3))
                        nc.scalar.activation(left_ext[32 * g:32 * g + 4, off:off + w],
                                             psm[0:4, 0:w], AF.Sigmoid,
                                             bias=wt['mxbl'][:], scale=1.0)
                        nc.scalar.activation(right_ext[32 * g:32 * g + 4, off:off + w],
                                             psm[32:36, 0:w], AF.Sigmoid,
                                             bias=wt['mxbr'][:], scale=1.0)

                    # u init for this group (Pool)
                    gr = slice(32 * g, 32 * g + 4)
                    nc.gpsimd.tensor_tensor(out=u_full[gr, :], in0=right_ext[gr, :],
                                            in1=left_ext[gr, 1:1 + GEXT], op=OP.add)
                    nc.gpsimd.tensor_tensor(out=u_full[gr, :], in0=u_full[gr, :],
                                            in1=curr1[gr, :], op=OP.mult)

                    # skew DMAs into scan layout (partition p = 16s + c)
                    for src_t, dst_t, width in ((u_full, u_t, FD),
                                                (right_ext, r_row, FD),
                                                (left_ext, l_row, LROW)):
                        src_pitch = src_t.shape[1]
                        for sp in range(4):
                            s_all = 4 * g + sp
                            in_ap = cap(src_t[:],
                                        [[src_pitch, 1], [128, C], [1, width]],
                                        offset=(32 * g + sp) * src_pitch)
                            nc.gpsimd.dma_start(
                                out=dst_t[16 * s_all:16 * s_all + C, 0:width],
                                in_=in_ap)

            # ------------------------ scan section ------------------------
            with tc.tile_pool(name="scanp", bufs=1) as scanp:
                a_t = scanp.tile([128, FD], f32, tag="a_t")
                b_t = scanp.tile([128, FD], f32, tag="b_t")
                pbuf = [scanp.tile([128, RH, FD], f32, name=f"pbuf{i}", tag=f"pbuf{i}")
                        for i in range(2)]

                def build_band(ph, r0, r1):
                    d0 = 1 + RH * ph + r0
                    nsub = r1 - r0
                    pb = pbuf[ph % 2]
                    # width needed for rows r0..r1: W(r) = FD - r; use FD - r0
                    wband = FD - r0
                    lsk = cap(l_row[:], [[LROW, 128], [1, nsub], [1, wband]],
                              offset=d0)
                    rbc = cap(r_row[:], [[FD, 128], [0, nsub], [1, wband]],
                              offset=0)
                    sl = pb[:, r0:r1, 0:wband]
                    nc.gpsimd.tensor_tensor(out=sl, in0=lsk, in1=rbc, op=OP.add)
                    nc.vector.reciprocal_approx_fast(out=sl, in_=sl)
                    nc.gpsimd.tensor_tensor(out=sl, in0=sl, in1=rbc, op=OP.mult)

                def build_phase(ph, nbands=2):
                    nsub = min(RH, NSTEP - RH * ph)
                    bounds = [nsub * i // nbands for i in range(nbands + 1)]
                    for i in range(nbands):
                        build_band(ph, bounds[i], bounds[i + 1])
                    return pbuf[ph % 2]

                pb_cur = build_phase(0, nbands=4)
                for ph in range(NPH):
                    nsub = min(RH, NSTEP - RH * ph)
                    pb_next = build_phase(ph + 1) if ph + 1 < NPH else None
                    X = XSPLIT
                    for r in range(nsub):
                        W = FD - r
                        x1 = min(W, X + 1)
                        x2 = min(W - 1, X)
                        nc.vector.tensor_tensor(out=a_t[:, 0:x1],
                                                in0=pb_cur[:, r, 0:x1],
                                                in1=u_t[:, 0:x1], op=OP.mult)
                        if W > x1:
                            nc.gpsimd.tensor_tensor(out=a_t[:, x1:W],
                                                    in0=pb_cur[:, r, x1:W],
                                                    in1=u_t[:, x1:W], op=OP.mult)
                        nc.vector.tensor_tensor(out=b_t[:, 0:x2],
                                                in0=u_t[:, 0:x2],
                                                in1=a_t[:, 0:x2],
                                                op=OP.subtract)
                        if W - 1 > x2:
                            nc.gpsimd.tensor_tensor(out=b_t[:, x2:W - 1],
                                                    in0=u_t[:, x2:W - 1],
                                                    in1=a_t[:, x2:W - 1],
                                                    op=OP.subtract)
                        nc.vector.tensor_tensor(out=u_t[:, 0:x2],
                                                in0=b_t[:, 0:x2],
                                                in1=a_t[:, 1:x2 + 1], op=OP.add)
                        if W - 1 > x2:
                            nc.gpsimd.tensor_tensor(out=u_t[:, x2:W - 1],
                                                    in0=b_t[:, x2:W - 1],
                                                    in1=a_t[:, x2 + 1:W],
                                                    op=OP.add)
                    if ph + 1 < NPH:
                        # halo: u[16s+c, 128:192] <- u[16s+c+1, 0:64], c<15
                        for s_all in range(S):
                            nc.gpsimd.dma_start(
                                out=u_t[16 * s_all:16 * s_all + 15, 128:FD],
                                in_=u_t[16 * s_all + 1:16 * s_all + 16, 0:RH])
                        pb_cur = pb_next

                # fixup: y = const^NSTEP * u / (right + left[511+j])
                fin = scanp.tile([128, 128], f32, tag="fin")
                fin2 = scanp.tile([128, 128], f32, tag="fin2")
                nc.vector.tensor_tensor(out=fin[:], in0=r_row[:, 0:128],
                                        in1=l_row[:, 511:511 + 128], op=OP.add)
                nc.vector.reciprocal_approx_fast(out=fin2[:], in_=fin[:])
                nc.vector.tensor_tensor(out=fin[:], in0=fin2[:],
                                        in1=u_t[:, 0:128], op=OP.mult)
                cpow = float(const_val) ** NSTEP
                nc.vector.tensor_scalar_mul(fin2[:], fin[:], cpow)
                for s_all in range(S):
                    nc.sync.dma_start(out=yout[s_all:s_all + 1, 0:1536],
                                      in_=fin2[16 * s_all:16 * s_all + 12, :])
                    nc.sync.dma_start(out=yout[s_all:s_all + 1, 1536:1537],
                                      in_=fin2[16 * s_all + 12:16 * s_all + 13, 0:1])

    nc.compile()
    return nc


_CACHE = {}


def _get_program(const_val):
    key = round(float(const_val), 8)
    if key not in _CACHE:
        _CACHE[key] = build_program(const_val)
    return _CACHE[key]


def make_in_maps(inputs):
    inputs = {k: np.asarray(v) for k, v in inputs.items()}
    wpack = _pack_weights(inputs)
    in_maps = []
    for core in range(NCORES):
        sl = slice(core * S, core * S + S)
        m = {'signal': np.ascontiguousarray(inputs['signal'][sl]),
             'seq': np.ascontiguousarray(inputs['seq'][sl]),
             'curr': np.ascontiguousarray(inputs['curr_diag'][sl])}
        m.update(wpack)
        in_maps.append(m)
    return in_maps


def kernel(**inputs):
    const_val = float(np.asarray(inputs['const']))
    nc = _get_program(const_val)
    in_maps = make_in_maps(inputs)
    from concourse.bass_utils import run_bass_kernel_spmd
    res = run_bass_kernel_spmd(nc, in_maps, list(range(NCORES)))
    out = np.concatenate([res.results[i]['y'] for i in range(NCORES)], axis=0)
    return out.astype(np.float32)



# revision 13
# speedup vs baseline: 1.0000x; 1.0000x over previous
"""DLEM kernel for Trainium2, 8 NeuronCores, data-parallel over batch.

Pipeline per core (8 samples):
  1. Conv section on the PE (float32r matmuls):
     conv0(160->128) and conv1(96->96) per sample, conv2(64->64) 2-sample
     block-diag, conv3(32->32) + all transposed convs 4-sample block-diag,
     mixer folded into 4 sparse [128,8] matmuls. PSUM->SBUF evacuation with
     fused bias+relu on ACT/DVE.
  2. DLEM scan rewritten as a linear 2-term recurrence without division:
         u_{d+1}[j] = u_d[j] - a_d[j] + a_d[j+1],  a_d = P_d * u_d,
         P(d,j) = right[j] / (right[j] + left[d+j])
     (u = curr * den / const^{d-1}; const and all divisions fold into the
     P coefficients and one final fixup.)
     Layout: 128 partitions = (sample s)*16 + chunk c, 16 chunks of 128
     positions + 32-deep halo refreshed per phase by a PE shift-matrix
     matmul evacuated on ACT.  Scan state is fp16 (DVE 2x mode); each step
     is 6 column-split DVE ops (L/R halves) so consecutive engine ops are
     never 1-back dependent and run back-to-back.  P coefficients are built
     one phase ahead on Pool+ACT via P = sigmoid(ln r - ln l): one fp16
     subtract (Pool) and one Sigmoid (ACT) per band.
"""
import os
import sys

for _p in ("/opt/trn_rl_repo", "/root/.axon_site/_ro/trn_rl_repo"):
    if os.path.isdir(_p) and _p not in sys.path:
        sys.path.insert(0, _p)

import numpy as np

B = 64
NCORES = 8
S = B // NCORES          # samples per core
N = 2048
EPI, SEQD, CPR = 128, 32, 32
NSTEP = 510              # d = 1..510
RH = 32                  # halo depth / phase length
FD = 128 + RH            # scan tile free dim (160)
C = 16                   # chunks per sample
NPH = (NSTEP + RH - 1) // RH   # 16 phases (last = 30 steps)
LEXT = 2700              # left_ext width
GEXT = C * 128 + RH      # 2080, right/u width
LROW = 672               # left row width (needs <= 671)

F32R = True              # use float32r (1 cyc/row) matmuls

_LBR = (2046, 2044, 2042, 2040)   # branch initial lengths
_T0S = (3, 4, 5, 6)               # branch initial data col offsets


# ---------------------------------------------------------------- weights --
def _pack_weights(inp):
    """All numpy-side weight packing. lhsT tensors are (K, 3, M)."""
    d = {}
    w0 = inp['conv0_w']            # (128, 160, 3)
    perm0 = np.concatenate([np.arange(32, 128), np.arange(0, 32)])
    d['c0A'] = np.ascontiguousarray(np.transpose(w0[perm0][:, :128, :], (1, 2, 0)))
    # c0B: K = (k, seq-ch) stacked 96 rows for the shift-stacked seq tile
    c0b_stk = np.zeros((96, 128), np.float32)
    for k in range(3):
        c0b_stk[32 * k:32 * k + 32, :] = w0[perm0][:, 128:, k].T
    d['c0B'] = c0b_stk
    d['c0bl'] = inp['conv0_b'][0:32].reshape(32, 1)
    d['c0bh'] = inp['conv0_b'][32:128].reshape(96, 1)

    w1 = inp['conv1_w']            # (96, 96, 3)
    perm1 = np.concatenate([np.arange(32, 96), np.arange(0, 32)])
    d['c1'] = np.ascontiguousarray(np.transpose(w1[perm1], (1, 2, 0)))
    d['c1bl'] = inp['conv1_b'][0:32].reshape(32, 1)
    d['c1bh'] = inp['conv1_b'][32:96].reshape(64, 1)

    w2 = inp['conv2_w']            # (64, 64, 3)
    l2 = np.zeros((128, 3, 128), np.float32)
    for k in range(3):
        t = w2[:, :, k].T          # (ci, co)
        # even sample: k-rows 0:64; odd: 64:128
        # out cols: 0:32 evenS co0:32 | 32:64 oddS co0:32
        #           64:96 evenS co32:64 | 96:128 oddS co32:64
        l2[0:64, k, 0:32] = t[:, 0:32]
        l2[0:64, k, 64:96] = t[:, 32:64]
        l2[64:128, k, 32:64] = t[:, 0:32]
        l2[64:128, k, 96:128] = t[:, 32:64]
    d['c2'] = l2
    b2 = np.zeros((128, 1), np.float32)
    b2[0:32, 0] = inp['conv2_b'][0:32]
    b2[32:64, 0] = inp['conv2_b'][0:32]
    b2[64:96, 0] = inp['conv2_b'][32:64]
    b2[96:128, 0] = inp['conv2_b'][32:64]
    d['c2bl'] = b2[0:64]
    d['c2bh'] = b2[64:128]

    w3 = inp['conv3_w']            # (32, 32, 3)
    l3 = np.zeros((128, 3, 128), np.float32)
    for k in range(3):
        t = w3[:, :, k].T
        for sl in range(4):
            l3[32 * sl:32 * sl + 32, k, 32 * sl:32 * sl + 32] = t
    d['c3'] = l3
    d['c3b'] = np.tile(inp['conv3_b'], 4).reshape(128, 1)

    for i in range(4):
        w = inp[f'tconv{i}_w']     # (ci=32, co=32, 3) torch layout
        wf = np.flip(w, -1).transpose(1, 0, 2)   # (co, ci, 3)
        lt = np.zeros((128, 3, 128), np.float32)
        for k in range(3):
            t = wf[:, :, k].T      # (ci, co)
            for sl in range(4):
                lt[32 * sl:32 * sl + 32, k, 32 * sl:32 * sl + 32] = t
        d[f't{i}'] = lt
        d[f't{i}b'] = np.tile(inp[f'tconv{i}_b'], 4).reshape(128, 1)

    mw = inp['mixer_w'][:, :, 0]   # (2, 128)
    for i in range(4):
        m = np.zeros((128, 36), np.float32)
        for sl in range(4):
            m[32 * sl:32 * sl + 32, sl] = mw[0, 32 * i:32 * i + 32]      # left
            m[32 * sl:32 * sl + 32, 32 + sl] = mw[1, 32 * i:32 * i + 32]  # right
        d[f'mx{i}'] = m
    d['mxbl'] = np.full((4, 1), inp['mixer_b'][0], np.float32)
    d['mxbr'] = np.full((4, 1), inp['mixer_b'][1], np.float32)
    # partition +1 shift matrix for the scan halo: out[m] = in[m+1]
    shf = np.zeros((128, 128), np.float16)
    for m in range(127):
        shf[m + 1, m] = 1.0
    d['shf'] = shf
    return d


_WSHAPES = {'c0A': [128, 3, 128],
            'c0bl': [32, 1], 'c0bh': [96, 1], 'c0B': [96, 128],
            'c1': [96, 3, 96], 'c1bl': [32, 1], 'c1bh': [64, 1],
            'c2': [128, 3, 128], 'c2bl': [64, 1], 'c2bh': [64, 1],
            'c3': [128, 3, 128], 'c3b': [128, 1],
            'mxbl': [4, 1], 'mxbr': [4, 1]}
for _i in range(4):
    _WSHAPES[f't{_i}'] = [128, 3, 128]
    _WSHAPES[f't{_i}b'] = [128, 1]
    _WSHAPES[f'mx{_i}'] = [128, 36]
_WSHAPES['shf'] = [128, 128]


def coltiles(L, maxw=512):
    out, off = [], 0
    while off < L:
        w = min(maxw, L - off)
        out.append((off, w))
        off += w
    return out


# ---------------------------------------------------------------- program --
def build_program(const_val):
    import bass_rust
    import concourse.bacc as bacc
    import concourse.mybir as mybir
    from concourse.tile import TileContext

    f32 = mybir.dt.float32
    f32r = mybir.dt.float32r
    f16 = mybir.dt.float16
    AF = mybir.ActivationFunctionType
    OP = mybir.AluOpType

    def cap(ap, dims, offset=None):
        b = ap.copy()
        b.ap = bass_rust.VecI64Pair(dims)
        if offset is not None:
            b.offset = offset
        return b

    def wdtype(k):
        if k == 'shf':
            return f16
        if k.endswith('b') or 'bl' in k or 'bh' in k or k in ('mxbl', 'mxbr'):
            return f32
        return f32r

    nc = bacc.Bacc("TRN2", target_bir_lowering=False, debug=False,
                   num_devices=NCORES)

    sig = nc.declare_dram_parameter("signal", [S, EPI, N], f32r, isOutput=False)
    seq = nc.declare_dram_parameter("seq", [S, SEQD, N], f32r, isOutput=False)
    curr = nc.declare_dram_parameter("curr", [S, N - 1], f32, isOutput=False)
    wd = {k: nc.declare_dram_parameter(k, sh, wdtype(k), isOutput=False)
          for k, sh in _WSHAPES.items()}
    yout = nc.declare_dram_parameter("y", [S, 1537], f32, isOutput=True)

    L0, L1, L2, L3 = _LBR[0], _LBR[1], _LBR[2], _LBR[3]
    BW = 2052

    with TileContext(nc) as tc:
        with (tc.tile_pool(name="wp", bufs=1) as wp,
              tc.tile_pool(name="lrp", bufs=1) as lrp,
              tc.tile_pool(name="psp", bufs=2, space="PSUM") as psp,
              tc.tile_pool(name="hpp", bufs=2, space="PSUM") as hpp):

            wt = {}
            for k, sh in _WSHAPES.items():
                t = wp.tile(sh, wdtype(k), name=f"w_{k}", tag=f"w_{k}")
                nc.scalar.dma_start(out=t[:], in_=wd[k][:])
                wt[k] = t

            # long-lived scan-side tiles (coexist with conv tiles)
            left_ext = lrp.tile([36, LEXT], f32, tag="left_ext")    # g0 rows 0:4, g1 rows 32:36
            right_ext = lrp.tile([36, GEXT], f32, tag="right_ext")
            curr1 = lrp.tile([36, GEXT], f32, tag="curr1")
            u_t = lrp.tile([128, FD], f16, tag="u_t")
            c_row = lrp.tile([128, FD], f32, tag="c_row")
            r_row = lrp.tile([128, FD], f32, tag="r_row")
            l_row = lrp.tile([128, LROW], f32, tag="l_row")
            lnr_row = lrp.tile([128, FD], f16, tag="lnr_row")
            lnl_row = lrp.tile([128, LROW], f16, tag="lnl_row")
            rec_row = lrp.tile([128, 128], f32, tag="rec_row")
            # pbuf0 lives in the long-lived pool so phase 0 can prebuild
            # during the conv section; pbuf1 is allocated post-conv.
            pbuf0 = lrp.tile([128, RH, FD], f16, tag="pbuf0")
            nc.gpsimd.memset(left_ext[:], 1.0)
            nc.gpsimd.memset(right_ext[:], 1.0)
            nc.gpsimd.memset(curr1[:], 0.0)
            for g in range(2):
                nc.scalar.dma_start(out=curr1[32 * g:32 * g + 4, 0:N - 1],
                                  in_=curr[4 * g:4 * g + 4])

            def build_band(pb, ph, r0, r1, p0, pn):
                """P band for phase ph, step rows [r0, r1), partitions
                [p0, p0+pn): z = ln r - ln l (Pool), P = sigmoid(z) (ACT)."""
                d0 = 1 + RH * ph + r0
                nsub = r1 - r0
                wband = FD - r0
                lsk = cap(lnl_row[:], [[LROW, pn], [1, nsub], [1, wband]],
                          offset=p0 * LROW + d0)
                rbc = cap(lnr_row[:], [[FD, pn], [0, nsub], [1, wband]],
                          offset=p0 * FD)
                sl = pb[p0:p0 + pn, r0:r1, 0:wband]
                nc.gpsimd.tensor_tensor(out=sl, in0=rbc, in1=lsk,
                                        op=OP.subtract)
                nc.scalar.activation(sl, sl, AF.Sigmoid, bias=0.0, scale=1.0)

            # ------------------------ conv section ------------------------
            with (tc.tile_pool(name="actp", bufs=1) as actp,
                  tc.tile_pool(name="iop", bufs=2) as iop):
                for g in range(2):
                    b_init = [actp.tile([128, BW], f32r, name=f"binit{i}", tag=f"binit{i}")
                              for i in range(4)]
                    b_alt = [actp.tile([128, BW], f32r, name=f"balt{i}", tag=f"balt{i}")
                             for i in range(4)]
                    zsrc = wt['c3'][:, 0, 0:8]
                    for i in range(4):
                        for t in (b_init[i], b_alt[i]):
                            nc.scalar.activation(t[:, 0:8], zsrc, AF.Copy,
                                                 bias=0.0, scale=0.0)
                            nc.scalar.activation(t[:, BW - 8:BW], zsrc, AF.Copy,
                                                 bias=0.0, scale=0.0)

                    conv2in = [actp.tile([128, L1], f32r, name=f"c2in{p}", tag=f"c2in{p}")
                               for p in range(2)]
                    conv3in = actp.tile([128, L2], f32r, tag="c3in")

                    def evac(ci, dst, src, bias, act_first=True):
                        """relu(src + bias) -> dst, alternating ACT/DVE."""
                        if (ci % 2 == 0) == act_first:
                            nc.scalar.activation(dst, src, AF.Relu,
                                                 bias=bias, scale=1.0)
                        else:
                            nc.vector.tensor_scalar(dst, src, bias, 0.0,
                                                    OP.add, OP.max)

                    for sl in range(4):
                        s = 4 * g + sl
                        x0a = iop.tile([128, N], f32r, tag="x0a")
                        nc.sync.dma_start(out=x0a[:], in_=sig[s])
                        # shift-stacked seq tile: rows (k, c) = seq[s, c, f+k]
                        seq_stk = iop.tile([96, L0], f32r, tag="seq_stk")
                        for k in range(3):
                            nc.sync.dma_start(out=seq_stk[32 * k:32 * k + 32, :],
                                              in_=seq[s][:, k:k + L0])
                        rest1 = iop.tile([96, L0], f32r, tag="rest1")

                        # conv0: 160->128 (3 sig shifts + 1 stacked-seq matmul)
                        for ci, (off, w) in enumerate(coltiles(L0)):
                            ps = psp.tile([128, 512], f32, tag="ps")
                            for k in range(3):
                                nc.tensor.matmul(
                                    out=ps[:, 0:w], lhsT=(wt['c0A'][:, k, :]),
                                    rhs=(x0a[:, off + k:off + k + w]),
                                    start=(k == 0), stop=False)
                            nc.tensor.matmul(
                                out=ps[:, 0:w], lhsT=(wt['c0B'][:]),
                                rhs=(seq_stk[:, off:off + w]),
                                start=False, stop=True)
                            evac(ci, b_init[0][32 * sl:32 * sl + 32,
                                               3 + off:3 + off + w],
                                 ps[96:128, 0:w], wt['c0bl'][:])
                            evac(ci + 1, rest1[:, off:off + w],
                                 ps[0:96, 0:w], wt['c0bh'][:])

                        # conv1: 96->96
                        for ci, (off, w) in enumerate(coltiles(L1)):
                            ps1 = psp.tile([96, 512], f32, tag="ps1")
                            for k in range(3):
                                nc.tensor.matmul(
                                    out=ps1[:, 0:w], lhsT=(wt['c1'][:, k, :]),
                                    rhs=(rest1[:, off + k:off + k + w]),
                                    start=(k == 0), stop=(k == 2))
                            evac(ci, b_init[1][32 * sl:32 * sl + 32,
                                               4 + off:4 + off + w],
                                 ps1[64:96, 0:w], wt['c1bl'][:])
                            evac(ci + 1,
                                 conv2in[sl // 2][64 * (sl % 2):64 * (sl % 2) + 64,
                                                  off:off + w],
                                 ps1[0:64, 0:w], wt['c1bh'][:])

                    # conv2 per pair (2 samples block-diag)
                    for p in range(2):
                        for ci, (off, w) in enumerate(coltiles(L2)):
                            ps = psp.tile([128, 512], f32, tag="ps")
                            for k in range(3):
                                nc.tensor.matmul(
                                    out=ps[:, 0:w], lhsT=(wt['c2'][:, k, :]),
                                    rhs=(conv2in[p][:, off + k:off + k + w]),
                                    start=(k == 0), stop=(k == 2))
                            evac(ci, b_init[2][64 * p:64 * p + 64,
                                               5 + off:5 + off + w],
                                 ps[0:64, 0:w], wt['c2bl'][:])
                            evac(ci + 1, conv3in[64 * p:64 * p + 64, off:off + w],
                                 ps[64:128, 0:w], wt['c2bh'][:])

                    # conv3 (4-sample block-diag)
                    for ci, (off, w) in enumerate(coltiles(L3)):
                        ps = psp.tile([128, 512], f32, tag="ps")
                        for k in range(3):
                            nc.tensor.matmul(
                                out=ps[:, 0:w], lhsT=(wt['c3'][:, k, :]),
                                rhs=(conv3in[:, off + k:off + k + w]),
                                start=(k == 0), stop=(k == 2))
                        evac(ci, b_init[3][:, 6 + off:6 + off + w],
                             ps[:, 0:w], wt['c3b'][:])

                    # transposed conv chains (4-sample block-diag)
                    finals = []
                    for i in range(4):
                        t0, L = _T0S[i], _LBR[i]
                        src, dst = b_init[i], b_alt[i]
                        for ti in range(3 - i, 4):
                            Lo = L + 2
                            for ci, (off, w) in enumerate(coltiles(Lo)):
                                ps = psp.tile([128, 512], f32, tag="ps")
                                for k in range(3):
                                    nc.tensor.matmul(
                                        out=ps[:, 0:w],
                                        lhsT=(wt[f't{ti}'][:, k, :]),
                                        rhs=(src[:, t0 - 2 + off + k:
                                                 t0 - 2 + off + k + w]),
                                        start=(k == 0), stop=(k == 2))
                                evac(ci, dst[:, t0 - 1 + off:t0 - 1 + off + w],
                                     ps[:, 0:w], wt[f't{ti}b'][:])
                            src, dst = dst, src
                            t0, L = t0 - 1, L + 2
                        finals.append(src)   # final 2048 cols at [2, 2050)

                    # mixer + sigmoid -> left/right rows directly
                    for (off, w) in coltiles(N):
                        psm = psp.tile([36, 512], f32, tag="psm")
                        for i in range(4):
                            nc.tensor.matmul(
                                out=psm[:, 0:w], lhsT=(wt[f'mx{i}'][:]),
                                rhs=(finals[i][:, 2 + off:2 + off + w]),
                                start=(i == 0), stop=(i == 3))
                        nc.scalar.activation(left_ext[32 * g:32 * g + 4, off:off + w],
                                             psm[0:4, 0:w], AF.Sigmoid,
                                             bias=wt['mxbl'][:], scale=1.0)
                        nc.scalar.activation(right_ext[32 * g:32 * g + 4, off:off + w],
                                             psm[32:36, 0:w], AF.Sigmoid,
                                             bias=wt['mxbr'][:], scale=1.0)

                    # skew DMAs into scan layout (partition p = 16s + c),
                    # per sample (dim0 count 1 + offset: the only DMA AP
                    # form whose extent the tile dep-tracker gets right),
                    # spread across three idle queues
                    for src_t, dst_t, width, eng in (
                            (curr1, c_row, FD, nc.scalar),
                            (right_ext, r_row, FD, nc.sync),
                            (left_ext, l_row, LROW, nc.gpsimd)):
                        sp_pitch = src_t.shape[1]
                        for sp in range(4):
                            s_all = 4 * g + sp
                            in_ap = cap(src_t[:],
                                        [[sp_pitch, 1], [128, C], [1, width]],
                                        offset=(32 * g + sp) * sp_pitch)
                            eng.dma_start(
                                out=dst_t[C * s_all:C * s_all + C, 0:width],
                                in_=in_ap)

                    # log rows for this group (ACT) + prebuild phase-0 P
                    # bands on Pool/ACT while the other group's convs run
                    p0 = 64 * g
                    nc.scalar.activation(lnr_row[p0:p0 + 64, :],
                                         r_row[p0:p0 + 64, :], AF.Ln,
                                         bias=0.0, scale=1.0)
                    nc.scalar.activation(lnl_row[p0:p0 + 64, :],
                                         l_row[p0:p0 + 64, :], AF.Ln,
                                         bias=0.0, scale=1.0)
                    nb0 = 1 if g == 0 else 8
                    bounds = [RH * i // nb0 for i in range(nb0 + 1)]
                    for i in range(nb0):
                        build_band(pbuf0, 0, bounds[i], bounds[i + 1], p0, 64)

            # ------------------------ scan section ------------------------
            with tc.tile_pool(name="scanp", bufs=1) as scanp:
                a_t = scanp.tile([128, FD], f16, tag="a_t")
                b_t = scanp.tile([128, FD], f16, tag="b_t")
                pbuf1 = scanp.tile([128, RH, FD], f16, tag="pbuf1")
                pbuf = [pbuf0, pbuf1]
                fin = scanp.tile([128, 128], f32, tag="fin")

                # fixup reciprocal (independent of u): rec = 1/(r + l[511+j])
                nc.vector.tensor_tensor(out=rec_row[:], in0=r_row[:, 0:128],
                                        in1=l_row[:, 511:511 + 128], op=OP.add)
                nc.vector.reciprocal_approx_fast(out=rec_row[:], in_=rec_row[:])

                # u init: u_1 = curr * (right + left[1+j]), clobbers r_row
                nc.vector.tensor_tensor(out=r_row[:, 0:FD], in0=r_row[:, 0:FD],
                                        in1=l_row[:, 1:FD + 1], op=OP.add)
                nc.vector.tensor_tensor(out=r_row[:, 0:FD], in0=r_row[:, 0:FD],
                                        in1=c_row[:], op=OP.mult)
                nc.vector.tensor_scalar_add(u_t[:], r_row[:, 0:FD], 0.0)

                def build_phase(ph, nbands=2):
                    nsub = min(RH, NSTEP - RH * ph)
                    bounds = [nsub * i // nbands for i in range(nbands + 1)]
                    for i in range(nbands):
                        build_band(pbuf[ph % 2], ph, bounds[i], bounds[i + 1],
                                   0, 128)
                    return pbuf[ph % 2]

                XM = 72
                pb_cur = pbuf0
                for ph in range(NPH):
                    nsub = min(RH, NSTEP - RH * ph)
                    pb_next = (build_phase(ph + 1,
                                           nbands=4 if ph < 2 else 2)
                               if 1 <= ph + 1 < NPH else None)
                    for r in range(nsub):
                        W = FD - r
                        nc.vector.tensor_tensor(out=a_t[:, 0:XM],
                                                in0=pb_cur[:, r, 0:XM],
                                                in1=u_t[:, 0:XM], op=OP.mult)
                        nc.vector.tensor_tensor(out=a_t[:, XM:W],
                                                in0=pb_cur[:, r, XM:W],
                                                in1=u_t[:, XM:W], op=OP.mult)
                        nc.vector.tensor_tensor(out=b_t[:, 0:XM],
                                                in0=u_t[:, 0:XM],
                                                in1=a_t[:, 0:XM],
                                                op=OP.subtract)
                        nc.vector.tensor_tensor(out=b_t[:, XM:W - 1],
                                                in0=u_t[:, XM:W - 1],
                                                in1=a_t[:, XM:W - 1],
                                                op=OP.subtract)
                        nc.vector.tensor_tensor(out=u_t[:, 0:XM],
                                                in0=b_t[:, 0:XM],
                                                in1=a_t[:, 1:XM + 1], op=OP.add)
                        nc.vector.tensor_tensor(out=u_t[:, XM:W - 1],
                                                in0=b_t[:, XM:W - 1],
                                                in1=a_t[:, XM + 1:W], op=OP.add)
                    if ph + 1 < NPH:
                        # halo u[p, 128:160] <- u[p+1, 0:32]: PE shift matmul,
                        # evacuated on ACT so the DVE queue never stalls
                        hps = hpp.tile([128, RH], f32, tag="hps")
                        nc.tensor.matmul(out=hps[:, 0:RH], lhsT=wt['shf'][:],
                                         rhs=u_t[:, 0:RH],
                                         start=True, stop=True)
                        nc.scalar.activation(u_t[:, 128:FD], hps[:, 0:RH],
                                             AF.Copy, bias=0.0, scale=1.0)
                        pb_cur = pbuf[(ph + 1) % 2]

                # fixup: y = const^NSTEP * u * rec
                cpow = float(const_val) ** NSTEP
                nc.vector.tensor_scalar_add(fin[:], u_t[:, 0:128], 0.0)
                nc.vector.tensor_tensor(out=fin[:], in0=fin[:],
                                        in1=rec_row[:], op=OP.mult)
                nc.vector.tensor_scalar_mul(fin[:], fin[:], cpow)
                # output DMAs, spread across four queues
                engs = (nc.sync, nc.scalar, nc.gpsimd)
                for s_all in range(S):
                    eng = engs[s_all % 3]
                    eng.dma_start(out=yout[s_all:s_all + 1, 0:1536],
                                  in_=fin[16 * s_all:16 * s_all + 12, :])
                    eng.dma_start(out=yout[s_all:s_all + 1, 1536:1537],
                                  in_=fin[16 * s_all + 12:16 * s_all + 13, 0:1])

    nc.compile()
    return nc


_CACHE = {}


def _get_program(const_val):
    key = round(float(const_val), 8)
    if key not in _CACHE:
        _CACHE[key] = build_program(const_val)
    return _CACHE[key]


def make_in_maps(inputs):
    inputs = {k: np.asarray(v) for k, v in inputs.items()}
    wpack = _pack_weights(inputs)
    in_maps = []
    for core in range(NCORES):
        sl = slice(core * S, core * S + S)
        m = {'signal': np.ascontiguousarray(inputs['signal'][sl]),
             'seq': np.ascontiguousarray(inputs['seq'][sl]),
             'curr': np.ascontiguousarray(inputs['curr_diag'][sl])}
        m.update(wpack)
        in_maps.append(m)
    return in_maps


def kernel(**inputs):
    const_val = float(np.asarray(inputs['const']))
    nc = _get_program(const_val)
    in_maps = make_in_maps(inputs)
    from concourse.bass_utils import run_bass_kernel_spmd
    res = run_bass_kernel_spmd(nc, in_maps, list(range(NCORES)))
    out = np.concatenate([res.results[i]['y'] for i in range(NCORES)], axis=0)
    return out.astype(np.float32)
